# revision 1
# baseline (speedup 1.0000x reference)
"""Trainium2 Bass kernel for nn_MixedAttnHeadEmbed (mixed-head-config attention).

Math (per batch b):
  Two attention configs share q_m/k_m/v_m [B,T,2048]:
    A: h=8  heads, d_max=256, mixing e in {1024,2048} -> d in {128,256}, weights w0,w1
    B: h=16 heads, d_max=128, mixing e in {1024,2048} -> d in {64,128},  weights w2,w3
  Each config: per-head q/k slices are RoPE'd, weight-summed (padded to d_max),
  GQA (8 kv heads), causal softmax attention; outputs of both configs sum.

Sharding: 8 cores = 4 batches x 2 shards. Shard s owns A-heads [4s,4s+4) and
B-heads [8s,8s+8) -> both write output columns [1024s, 1024s+1024) which are
summed on device; per-core output is the transposed block outT [1024, T].

Device layout: scores computed transposed (sT[k,q], k on partitions) so the
softmax'd weights feed the y^T matmul with no on-chip transposes; softmax is
max-free (scores are provably < 2 for this problem family; exp is safe in
fp32) with the denominator from an all-ones stationary matmul.
"""

import math
from contextlib import ExitStack
from dataclasses import dataclass

import numpy as np

import concourse.bass as bass
import concourse.mybir as mybir
import concourse.tile as tile
from concourse import bacc

F32 = mybir.dt.float32
F32R = mybir.dt.float32r
NEG = -1e9
P = 128


@dataclass(frozen=True)
class KCfg:
    T: int = 1024       # sequence length
    NA: int = 4         # config-A heads per core (d_max=256)
    NB: int = 8         # config-B heads per core (d_max=128); must be 2*NA
    REG: int = 512      # psum region width (<=512)

    @property
    def TK(self):
        return self.T // P

    @property
    def NREG(self):
        return self.T // self.REG

    @property
    def NKVB(self):
        return self.NB // 2

    @property
    def ROWS(self):
        return self.NA * 256  # == NB * 128 output rows per core


FULL = KCfg()


def _in_specs(cfg: KCfg):
    T = cfg.T
    return {
        "qT1": (cfg.NA * 128, T),
        "qT2": (cfg.NA * 256, T),
        "kTa1": (cfg.NA * 128, T),
        "kTa2": (cfg.NA * 256, T),
        "kTb1": (cfg.NKVB * 64, T),
        "kTb2": (cfg.NKVB * 128, T),
        "va1": (T, cfg.NA * 128),
        "va2": (T, cfg.NA * 256),
        "vb1": (T, cfg.NKVB * 64),
        "vb2": (T, cfg.NKVB * 128),
        "ca1": (128, T), "sa1": (128, T),
        "ca2": (256, T), "sa2": (256, T),
        "cb1": (128, T), "sb1": (128, T),
        "cb2": (128, T), "sb2": (128, T),
        "wvec": (P, 4),
    }


class _EngPick:
    """Static load balancer across DVE / GPSIMD / ACT.

    units: 1.0 ~ one [.,1024] fp32 pass. Cost-model calibration: DVE and
    Pool run TT at ~1 elem/lane/cycle (fp32 has no DVE fast mode); ACT can
    only take single-input copies, and it also carries all the exps (those
    are tallied in via act())."""

    GP_W = 1.05   # tuned: bias work toward pool
    ACT_W = 1.5

    def __init__(self, nc):
        self.nc = nc
        self.load = {"dve": 0.0, "pool": 0.0, "act": 0.0}

    def dve(self, units=1.0):
        self.load["dve"] += units
        return self.nc.vector

    def act(self, units=1.0):
        self.load["act"] += units * self.ACT_W
        return self.nc.scalar

    def tt(self, units=1.0):
        """2-input sbuf op: DVE or GPSIMD."""
        if self.load["dve"] + units <= self.load["pool"] + self.GP_W * units:
            return self.dve(units)
        self.load["pool"] += self.GP_W * units
        return self.nc.gpsimd

    def copy(self, dst, src, units=1.0):
        """1-input copy: any of the three engines."""
        costs = {"dve": units, "pool": self.GP_W * units,
                 "act": self.ACT_W * units}
        eng = min(costs, key=lambda k: self.load[k] + costs[k])
        self.load[eng] += costs[eng]
        if eng == "act":
            self.nc.scalar.copy(dst, src)
        elif eng == "pool":
            self.nc.gpsimd.tensor_copy(dst, src)
        else:
            self.nc.vector.tensor_copy(dst, src)


def build_program(cfg: KCfg = FULL):
    # Bacc (not plain Bass): its compile() runs generate_event_semaphores,
    # which splits multi-wait sync_infos — TRN2 allows 1 wait per instruction.
    nc = bacc.Bacc("TRN2", target_bir_lowering=False)
    T, TK, REG, NREG = cfg.T, cfg.TK, cfg.REG, cfg.NREG
    RPB = REG // P  # k-chunks per region

    D = {}
    for name, shape in _in_specs(cfg).items():
        D[name] = nc.declare_dram_parameter(name, list(shape), F32, isOutput=False)
    outT = nc.declare_dram_parameter("outT", [cfg.ROWS, T], F32, isOutput=True)
    RB = cfg.ROWS // P

    mult, add = mybir.AluOpType.mult, mybir.AluOpType.add

    with ExitStack() as ctx:
        tc = ctx.enter_context(tile.TileContext(nc))
        const = ctx.enter_context(tc.tile_pool(name="const", bufs=1))
        rawp = ctx.enter_context(tc.tile_pool(name="raw", bufs=2))
        mixp = ctx.enter_context(tc.tile_pool(name="mix", bufs=2))
        scr = ctx.enter_context(tc.tile_pool(name="scr", bufs=1))
        ppool = ctx.enter_context(tc.tile_pool(name="pp", bufs=3))
        accp = ctx.enter_context(tc.tile_pool(name="acc", bufs=1))
        normp = ctx.enter_context(tc.tile_pool(name="norm", bufs=1))
        spsum = ctx.enter_context(tc.tile_pool(name="spsum", bufs=2, space="PSUM"))
        ypsum = ctx.enter_context(tc.tile_pool(name="ypsum", bufs=1, space="PSUM"))
        dpsum = ctx.enter_context(tc.tile_pool(name="dpsum", bufs=1, space="PSUM"))

        pick = _EngPick(nc)

        # ---- constants ----
        ones_f = const.tile([P, P], F32, name="ones_f")
        nc.vector.memset(ones_f, 1.0)
        ones = const.tile([P, P], F32R)
        nc.vector.tensor_copy(ones, ones_f)  # rounds to f32r for the matmul
        dmask = const.tile([P, P], F32)
        nc.gpsimd.memset(dmask, 0.0)
        # dmask[k, q] = 0 where q >= k else NEG  (transposed causal diag block)
        nc.gpsimd.affine_select(
            out=dmask, in_=dmask, compare_op=mybir.AluOpType.is_ge,
            fill=NEG, base=0, pattern=[[1, P]], channel_multiplier=-1,
        )
        tabs = {}
        for nm in ("ca1", "sa1", "ca2", "sa2", "cb1", "sb1", "cb2", "sb2"):
            rows = _in_specs(cfg)[nm][0]
            tl = const.tile([P, rows // P, T], F32, name=nm, tag=nm)
            tabs[nm] = tl
            nc.sync.dma_start(out=tl, in_=D[nm].rearrange("(c p) t -> p c t", p=P))
        wv = const.tile([P, 4], F32)
        nc.sync.dma_start(out=wv, in_=D["wvec"][:, :])

        outacc = accp.tile([P, RB, T], F32)

        def halfmul(dst, src, tab, half, base=0, rows=P):
            """dst[base:base+rows][j] = src[sigma(j)] * tab_math[j], where
            sigma swaps halves of size `half` within each 2*half group.

            tab is the HOST-SIGMA-PERMUTED signed sin table, so the multiply
            is same-base (u = src*tab) and the rotation becomes 1-input
            cross-base copies (the only cross-partition-base op trn2 allows).
            """
            u = scr.tile([P, T], F32, tag="xbt", name="xbt")
            usl = u[base:base + rows, :]
            pick.tt(1.0).tensor_tensor(usl, src, tab, mult)
            for g in range(rows // (2 * half)):
                b0 = base + 2 * half * g
                pick.copy(dst[b0:b0 + half, :], u[b0 + half:b0 + 2 * half, :], 1.0)
                pick.copy(dst[b0 + half:b0 + 2 * half, :], u[b0:b0 + half, :], 1.0)

        def xb_add(dst, src, units):
            """dst += src across partition bases (copy to re-base, then add)."""
            n = src.shape[0]
            tmp = scr.tile([P, T], F32, tag="xbt2", name="xbt2")
            view = tmp[0:n, :]
            pick.copy(view, src, units)
            pick.tt(units).tensor_tensor(dst, dst, view, add)

        def mix_qk_A(out, x1, x2, c1, s1, c2, s2):
            """out [P,2,T] = rope-mix for a config-A head.
            x1 [P,T] (d=128 slice), x2 [P,2,T] (d=256 slice).
            s1 is sigma64-permuted; s2 is the plain signed sin table."""
            t1 = scr.tile([P, T], F32, tag="t1")
            t2 = scr.tile([P, T], F32, tag="t2")
            # dc0: x2t0*c2_0 + x2t1*s2_0 + x1*c1 + shift64(x1)*s1
            pick.tt().tensor_tensor(out[:, 0, :], x2[:, 0, :], c2[:, 0, :], mult)
            pick.tt().tensor_tensor(t1, x2[:, 1, :], s2[:, 0, :], mult)
            pick.tt().tensor_tensor(out[:, 0, :], out[:, 0, :], t1, add)
            pick.tt().tensor_tensor(t1, x1, c1[:, 0, :], mult)
            halfmul(t2, x1, s1[:, 0, :], 64)
            pick.tt().tensor_tensor(t1, t1, t2, add)
            pick.tt().tensor_tensor(out[:, 0, :], out[:, 0, :], t1, add)
            # dc1: x2t1*c2_1 + x2t0*s2_1
            pick.tt().tensor_tensor(out[:, 1, :], x2[:, 1, :], c2[:, 1, :], mult)
            pick.tt().tensor_tensor(t1, x2[:, 0, :], s2[:, 1, :], mult)
            pick.tt().tensor_tensor(out[:, 1, :], out[:, 1, :], t1, add)

        def mix_qk_B_pair(out, x1p, x2p, c1, s1, c2, s2):
            """out [P,2,T]: B-head pair. out[:,j,:] for heads (2p+j).
            x2p [P,2,T] (d=128 per head), x1p [P,T] packed pair (d=64 each).
            s2 sigma64-permuted; s1 sigma32-permuted."""
            t1 = scr.tile([P, T], F32, tag="t1")
            t2 = scr.tile([P, T], F32, tag="t2")
            for j in range(2):
                pick.tt().tensor_tensor(out[:, j, :], x2p[:, j, :], c2[:, 0, :], mult)
                halfmul(t1, x2p[:, j, :], s2[:, 0, :], 64)
                pick.tt().tensor_tensor(out[:, j, :], out[:, j, :], t1, add)
            # packed d=64 contributions for both heads of the pair
            pick.tt().tensor_tensor(t1, x1p, c1[:, 0, :], mult)
            halfmul(t2, x1p, s1[:, 0, :], 32)
            pick.tt().tensor_tensor(t1, t1, t2, add)
            pick.tt(1.0).tensor_tensor(out[0:64, 0, :], out[0:64, 0, :],
                                       t1[0:64, :], add)
            xb_add(out[0:64, 1, :], t1[64:128, :], 1.0)

        def subchunks(c):
            out = []
            for r in range(NREG):
                q0 = max(REG * r, P * c)
                q1 = REG * (r + 1)
                if q1 > q0:
                    out.append((r, q0, q1 - q0))
            return out

        def attn_head(qmixs, kmixs, vmix, blks, is_b):
            """qmixs/kmixs: per-d-chunk [P, T] APs; vmix [P, TK, ndc*P].

            Matmul operands are bitcast to float32r: full-rate PE streaming
            (fp32 proper runs at 1/4 rate) with near-fp32 accumulation."""
            ndc = len(qmixs)
            den = dpsum.tile([P, T], F32, tag="den")
            yts = [ypsum.tile([P, T], F32, tag=f"yt{i}", name=f"yt{i}")
                   for i in range(ndc)]
            for c in range(TK):
                for (r, q0, n) in subchunks(c):
                    last_c = min(TK, RPB * (r + 1)) - 1
                    sT = spsum.tile([P, REG], F32, tag="sT")
                    for dc in range(ndc):
                        nc.tensor.matmul(
                            sT[:, :n],
                            kmixs[dc][:, P * c:P * (c + 1)],
                            qmixs[dc][:, q0:q0 + n],
                            start=(dc == 0), stop=(dc == ndc - 1))
                    if q0 == P * c:  # diagonal block gets the causal mask
                        pick.dve(0.125).tensor_tensor(sT[:, :P], sT[:, :P],
                                                      dmask, add)
                    pt = ppool.tile([P, REG], F32R, tag="pT")
                    pick.act(n / 1024.0).activation(
                        pt[:, :n], sT[:, :n], mybir.ActivationFunctionType.Exp)
                    for dc in range(ndc):
                        nc.tensor.matmul(
                            yts[dc][:, q0:q0 + n],
                            vmix[:, c, P * dc:P * (dc + 1)],
                            pt[:, :n],
                            start=(c == 0), stop=(c == last_c))
                    nc.tensor.matmul(den[:, q0:q0 + n], ones,
                                     pt[:, :n],
                                     start=(c == 0), stop=(c == last_c))
            rec = normp.tile([P, T], F32, tag="rec")
            pick.dve(1.0).reciprocal(rec, den)
            for dc in range(ndc):
                blk = blks[dc]
                if not is_b:
                    pick.dve(1.0).tensor_tensor(outacc[:, blk, :], yts[dc][:, :],
                                                rec, mult)
                else:
                    tmp = normp.tile([P, T], F32, tag="btmp")
                    pick.dve(1.0).tensor_tensor(tmp, yts[dc][:, :], rec, mult)
                    pick.tt(1.0).tensor_tensor(outacc[:, blk, :],
                                               outacc[:, blk, :], tmp, add)
                    nc.sync.dma_start(out=outT[P * blk:P * (blk + 1), :],
                                      in_=outacc[:, blk, :])

        # ================= config A =================
        for h in range(cfg.NA):
            q1 = rawp.tile([P, T], F32, tag="rS")
            nc.sync.dma_start(out=q1, in_=D["qT1"][P * h:P * (h + 1), :])
            q2 = rawp.tile([P, 2, T], F32, tag="rD")
            nc.sync.dma_start(out=q2, in_=D["qT2"][256 * h:256 * (h + 1), :]
                              .rearrange("(c p) t -> p c t", p=P))
            qmix = mixp.tile([P, 2, T], F32R, tag="qmix")
            mix_qk_A(qmix, q1, q2, tabs["ca1"], tabs["sa1"], tabs["ca2"], tabs["sa2"])

            k1 = rawp.tile([P, T], F32, tag="rS")
            nc.sync.dma_start(out=k1, in_=D["kTa1"][P * h:P * (h + 1), :])
            k2 = rawp.tile([P, 2, T], F32, tag="rD")
            nc.sync.dma_start(out=k2, in_=D["kTa2"][256 * h:256 * (h + 1), :]
                              .rearrange("(c p) t -> p c t", p=P))
            kmix = mixp.tile([P, 2, T], F32R, tag="kmix")
            mix_qk_A(kmix, k1, k2, tabs["ca1"], tabs["sa1"], tabs["ca2"], tabs["sa2"])

            v1 = rawp.tile([P, TK, P], F32, tag="rv1")
            nc.sync.dma_start(out=v1, in_=D["va1"][:, P * h:P * (h + 1)]
                              .rearrange("(c p) d -> p c d", p=P))
            v2 = rawp.tile([P, TK, 2 * P], F32, tag="rv2")
            nc.sync.dma_start(out=v2, in_=D["va2"][:, 2 * P * h:2 * P * (h + 1)]
                              .rearrange("(c p) d -> p c d", p=P))
            vmix = mixp.tile([P, TK, 2 * P], F32R, tag="vmix")
            pick.dve(2.0).tensor_scalar_mul(vmix, v2, wv[:, 1:2])
            pick.dve(1.0).scalar_tensor_tensor(
                out=vmix[:, :, 0:P], in0=v1, scalar=wv[:, 0:1],
                in1=vmix[:, :, 0:P], op0=mult, op1=add)

            attn_head([qmix[:, 0, :], qmix[:, 1, :]],
                      [kmix[:, 0, :], kmix[:, 1, :]],
                      vmix, (2 * h, 2 * h + 1), is_b=False)

        # ================= config B =================
        for j in range(cfg.NKVB):  # kv head j serves B-heads (2j, 2j+1)
            k2 = rawp.tile([P, T], F32, tag="rS")
            nc.sync.dma_start(out=k2, in_=D["kTb2"][P * j:P * (j + 1), :])
            # packed pair of d=64 kv slices: kv (2*(j//2)), (2*(j//2)+1)
            k1p = rawp.tile([P, T], F32, tag="rS")
            jp = j // 2
            nc.sync.dma_start(out=k1p, in_=D["kTb1"][P * jp:P * (jp + 1), :])

            kmix = mixp.tile([P, T], F32R, tag="kmix")
            t1 = scr.tile([P, T], F32, tag="t1")
            pick.tt().tensor_tensor(kmix, k2, tabs["cb2"][:, 0, :], mult)
            halfmul(t1, k2, tabs["sb2"][:, 0, :], 64)
            pick.tt().tensor_tensor(kmix, kmix, t1, add)
            # d=64 part only on rows 0:64 (uses half of the packed pair tile)
            half = 0 if j % 2 == 0 else 64
            sl = slice(half, half + 64)
            ts = scr.tile([P, T], F32, tag="t2", name="ts")
            pick.tt().tensor_tensor(ts[sl, :], k1p[sl, :],
                                    tabs["cb1"][sl, 0, :], mult)
            tb = scr.tile([P, T], F32, tag="t3", name="tb")
            halfmul(tb, k1p[sl, :], tabs["sb1"][sl, 0, :], 32, base=half, rows=64)
            pick.tt().tensor_tensor(ts[sl, :], ts[sl, :], tb[sl, :], add)
            if half == 0:
                pick.tt().tensor_tensor(kmix[0:64, :], kmix[0:64, :], ts[sl, :], add)
            else:
                xb_add(kmix[0:64, :], ts[sl, :], 1.0)

            v2 = rawp.tile([P, TK, P], F32, tag="rv1")
            nc.sync.dma_start(out=v2, in_=D["vb2"][:, P * j:P * (j + 1)]
                              .rearrange("(c p) d -> p c d", p=P))
            v1 = rawp.tile([P, TK, 64], F32, tag="rv2")
            nc.sync.dma_start(out=v1, in_=D["vb1"][:, 64 * j:64 * (j + 1)]
                              .rearrange("(c p) d -> p c d", p=P))
            vmix = mixp.tile([P, TK, P], F32R, tag="vmix")
            pick.dve(1.0).tensor_scalar_mul(vmix, v2, wv[:, 3:4])
            pick.dve(0.5).scalar_tensor_tensor(
                out=vmix[:, :, 0:64], in0=v1, scalar=wv[:, 2:3],
                in1=vmix[:, :, 0:64], op0=mult, op1=add)

            # q pair for heads (2j, 2j+1)
            q2p = rawp.tile([P, 2, T], F32, tag="rD")
            nc.sync.dma_start(out=q2p, in_=D["qT2"][256 * j:256 * (j + 1), :]
                              .rearrange("(c p) t -> p c t", p=P))
            q1p = rawp.tile([P, T], F32, tag="rS")
            nc.sync.dma_start(out=q1p, in_=D["qT1"][P * j:P * (j + 1), :])
            qmixp = mixp.tile([P, 2, T], F32R, tag="qmix")
            mix_qk_B_pair(qmixp, q1p, q2p, tabs["cb1"], tabs["sb1"],
                          tabs["cb2"], tabs["sb2"])

            for hh in range(2):
                b = 2 * j + hh
                attn_head([qmixp[:, hh, :]], [kmix], vmix, (b,), is_b=True)

    nc.compile()
    return nc


# ---------------------------------------------------------------------------
# Host side
# ---------------------------------------------------------------------------

def _rope_tab(pos, d, f):
    """Transposed rope tables [d, T]: (f*cos, +-f*sin with rot sign folded)."""
    inv = 1.0 / (10000.0 ** (np.arange(0, d, 2, dtype=np.float32) / d))
    ang = inv[:, None] * pos[None, :].astype(np.float32)      # [d/2, T]
    ang = np.concatenate([ang, ang], 0)                        # [d, T]
    c = (f * np.cos(ang)).astype(np.float32)
    s = (f * np.sin(ang)).astype(np.float32)
    s[: d // 2] *= -1.0
    return c, s


def make_core_inputs(q, k, v, pos, weights, s, cfg: KCfg = FULL):
    """q,k,v: [T, 2048] for one batch; returns the per-core input dict."""
    T = cfg.T
    c = np.ascontiguousarray
    arrs = {
        "qT1": c(q[:, 512 * s:512 * s + 512].T),
        "qT2": c(q[:, 1024 * s:1024 * s + 1024].T),
        "kTa1": c(k[:, 512 * s:512 * s + 512].T),
        "kTa2": c(k[:, 1024 * s:1024 * s + 1024].T),
        "kTb1": c(k[:, 256 * s:256 * s + 256].T),
        "kTb2": c(k[:, 512 * s:512 * s + 512].T),
        "va1": c(v[:, 512 * s:512 * s + 512]),
        "va2": c(v[:, 1024 * s:1024 * s + 1024]),
        "vb1": c(v[:, 256 * s:256 * s + 256]),
        "vb2": c(v[:, 512 * s:512 * s + 512]),
    }
    fA = math.sqrt(1.0 / 16.0)
    fB = math.sqrt(1.0 / math.sqrt(128.0))
    ca1, sa1 = _rope_tab(pos, 128, fA * float(weights[0]))
    ca2, sa2 = _rope_tab(pos, 256, fA * float(weights[1]))
    cb1h, sb1h = _rope_tab(pos, 64, fB * float(weights[2]))
    cb2, sb2 = _rope_tab(pos, 128, fB * float(weights[3]))

    def sigma(tab, half):
        # swap halves of size `half` within each 2*half row group
        out = tab.reshape(-1, 2, half, tab.shape[-1])
        return np.ascontiguousarray(
            out[:, ::-1].reshape(tab.shape))

    sb1 = np.vstack([sb1h, sb1h])
    arrs.update({
        # sin tables used through within-tile rotations are stored
        # sigma-permuted (device computes u = x * s_sigma, then rotates u
        # via cross-base copies); sa2 (d=256) rotates across tiles and
        # stays in math order.
        "ca1": ca1, "sa1": sigma(sa1, 64), "ca2": ca2, "sa2": sa2,
        "cb1": np.vstack([cb1h, cb1h]), "sb1": sigma(sb1, 32),
        "cb2": cb2, "sb2": sigma(sb2, 64),
        "wvec": np.tile(np.asarray(weights, np.float32)[None, :], (P, 1)),
        # math-order copies for numpy models (not used by the device)
        "_m_sa1": sa1, "_m_sb1": sb1, "_m_sb2": sb2,
    })
    return arrs


_PROGRAM_CACHE = {}
TRACE = False
LAST_RESULT = None


def kernel(q_m, k_m, v_m, weights, attention_mask, position_ids):
    global LAST_RESULT
    from concourse.bass_utils import run_bass_kernel_spmd

    cfg = FULL
    q_m = np.asarray(q_m, np.float32)
    k_m = np.asarray(k_m, np.float32)
    v_m = np.asarray(v_m, np.float32)
    weights = np.asarray(weights, np.float32)
    attention_mask = np.asarray(attention_mask, np.float32)
    position_ids = np.asarray(position_ids)
    B, T, H = q_m.shape

    # the device program hardcodes the causal structure; verify it holds
    causal = np.where(np.tril(np.ones((T, T), bool)), 0.0, NEG).astype(np.float32)
    for b in range(B):
        assert np.array_equal(attention_mask[b, 0], causal), "non-causal mask"

    if "nc" not in _PROGRAM_CACHE:
        _PROGRAM_CACHE["nc"] = build_program(cfg)
    nc = _PROGRAM_CACHE["nc"]

    in_maps = []
    for b in range(B):
        for s in range(2):
            in_maps.append(make_core_inputs(
                q_m[b], k_m[b], v_m[b], position_ids[b], weights, s, cfg))
    res = run_bass_kernel_spmd(nc, in_maps, list(range(8)), trace=TRACE)
    LAST_RESULT = res
    out = np.zeros((B, T, H), np.float32)
    for b in range(B):
        for s in range(2):
            out[b, :, 1024 * s:1024 * s + 1024] = res.results[2 * b + s]["outT"].T
    return out



# revision 4
# speedup vs baseline: 1.2397x; 1.2397x over previous
"""Trainium2 Bass kernel for nn_MixedAttnHeadEmbed (mixed-head-config attention).

v2: bf16 end-to-end with [q,d]-layout outputs.

Math (per batch b): two attention configs share q_m/k_m/v_m [B,T,2048]:
  A: h=8  heads, d_max=256, mixing e in {1024,2048} -> d in {128,256}, w0,w1
  B: h=16 heads, d_max=128, mixing e in {1024,2048} -> d in {64,128},  w2,w3
Sharding: 8 cores = 4 batches x 2 shards; shard s owns A-heads [4s,4s+4) and
B-heads [8s,8s+8) -> output cols [1024s, 1024s+1024), written [T, 1024].

Device design notes:
 - Everything bf16 on SBUF (DVE 2x/4x fast modes, full-rate PE, half DMA);
   PSUM f32 only for matmul accumulation.
 - Weight folding moves all mix scalars into host-precomputed rope tables:
     qmA = fA^2 w0 (w0 P(r1q) + w1 R2q),   kmA = P(r1k) + (w1/w0) R2k
     qmB = fB^2 w3 (w3 R128(x) + w2 P(r64q)), kmB = r1k[g] + (w2/w3) P(r64k)
   where r1 = unscaled rope128 (k-side shared between A and B).
 - Rope rotations are free: sigma-permuted DATA copies ship from host, so
   rope(x) = x*c + xs*s with signed math-order sin tables (3 TT per tile).
 - Scores computed transposed sT[k,q] per k-chunk; causal mask applied ON THE
   PE (strict-tri(-1e9) @ I accumulated into the diagonal block); exp on ACT
   (one inst per chunk) into an SBUF bf16 pt [128, 8, T].
 - Phase 2 emits y in [q-part, d-free] via pt-stationary matmuls with an
   AUGMENTED V (ones column) so the softmax denominator is the last y column;
   normalize = tensor_scalar divide by that per-partition column (no
   reciprocal, no transposes, output lands in natural [T, H] layout).
 - A static engine balancer routes elementwise work DVE/Pool by modeled cost.
"""

import math
from contextlib import ExitStack
from dataclasses import dataclass

import numpy as np
import ml_dtypes

import concourse.bass as bass
import concourse.mybir as mybir
import concourse.tile as tile
from concourse import bacc

F32 = mybir.dt.float32
BF = mybir.dt.bfloat16
NPBF = ml_dtypes.bfloat16
NEG = -1e9
P = 128
T = 1024
TK = T // P


@dataclass(frozen=True)
class KCfg:
    pass


FULL = KCfg()

mult = mybir.AluOpType.mult
add = mybir.AluOpType.add
sub = mybir.AluOpType.subtract
div = mybir.AluOpType.divide
Exp = mybir.ActivationFunctionType.Exp


class _Pick:
    """Cost-model-accurate static balancer for DVE / Pool elementwise ops."""

    def __init__(self, nc):
        self.nc = nc
        self.load = {"dve": 0.0, "pool": 0.0}

    def _choose(self, cd, cp):
        if self.load["dve"] + cd <= self.load["pool"] + cp:
            self.load["dve"] += cd
            return self.nc.vector
        self.load["pool"] += cp
        return self.nc.gpsimd

    def tt(self, out, a, b, op, free, psum=False):
        cd = (1.0417 * free + 125) if psum else (0.52 * free + 60)
        eng = self._choose(cd, 0.8333 * free + 120)
        eng.tensor_tensor(out, a, b, op)

    def tsp(self, out, a, scal, op, free, psum=False):
        cd = (1.0417 * free + 125) if psum else (0.26 * free + 60)
        eng = self._choose(cd, 0.8333 * free + 120)
        eng.tensor_scalar(out=out, in0=a, scalar1=scal, scalar2=None, op0=op)

    def stt(self, out, a, scal, b, op0, op1, free, psum=False):
        cd = (1.0417 * free + 125) if psum else (1.0417 * free + 60)
        eng = self._choose(cd, 0.8333 * free + 120)
        eng.scalar_tensor_tensor(out=out, in0=a, scalar=scal, in1=b,
                                 op0=op0, op1=op1)

    def cp(self, dst, src, free, psum=False):
        cd = (1.0417 * free + 125) if psum else (0.26 * free + 60)
        eng = self._choose(cd, 0.8333 * free + 120)
        eng.tensor_copy(dst, src)


def build_program(cfg: KCfg = FULL):
    nc = bacc.Bacc("TRN2", target_bir_lowering=False)

    def dram(name, shape, dt=BF, out=False):
        return nc.declare_dram_parameter(name, list(shape), dt, isOutput=out)

    D = {
        "r1in": dram("r1in", (16, P, T)),      # qa1 qa1s ka1 ka1s (4 ch each)
        "tabr1": dram("tabr1", (4, P, T)),     # c1q s1q c1 s1 (signed)
        "d64q": dram("d64q", (64, 16, T)),     # (g,hh) data 0:8 | sigma32 8:16
        "d64k": dram("d64k", (64, 8, T)),      # g data 0:4 | sigma32 4:8
        "t64": dram("t64", (64, 4, T)),        # c64q s64q c64k s64k
        "qa2": dram("qa2", (8, P, T)),
        "ka2": dram("ka2", (8, P, T)),
        "tabm": dram("tabm", (6, P, T)),       # c2q s2q c2k s2k cBq sBq
        "vcat": dram("vcat", (T, 1792)),       # va1 512 | va2 1024 | vb1 256
        "consts": dram("consts", (2, P, P)),   # tri(NEG strict upper), iden
        "wvec": dram("wvec", (P, 4), dt=F32),
    }
    outQ = dram("outQ", (T, 1024), out=True)

    with ExitStack() as ctx:
        tc = ctx.enter_context(tile.TileContext(nc))
        pers = ctx.enter_context(tc.tile_pool(name="pers", bufs=1))
        pick = _Pick(nc)

        # ---------------- persistent tiles ----------------
        r1q = pers.tile([P, 4, T], BF, name="r1q")
        r1k = pers.tile([P, 4, T], BF, name="r1k")
        r64q = pers.tile([64, 8, T], BF, name="r64q")
        r64k = pers.tile([64, 4, T], BF, name="r64k")
        tabm = pers.tile([P, 6, T], BF, name="tabm")
        nc.sync.dma_start(out=tabm, in_=D["tabm"].rearrange("c p t -> p c t"))
        cst = pers.tile([P, 2, P], BF, name="cst")
        nc.sync.dma_start(out=cst, in_=D["consts"].rearrange("c p t -> p c t"))
        tri, iden = cst[:, 0, :], cst[:, 1, :]
        wv = pers.tile([P, 4], F32, name="wv")
        nc.sync.dma_start(out=wv, in_=D["wvec"][:, :])
        qa2t = pers.tile([P, 8, T], BF, name="qa2t")
        nc.sync.dma_start(out=qa2t, in_=D["qa2"].rearrange("c p t -> p c t"))
        vcat = pers.tile([P, TK, 1792], BF, name="vcat")
        nc.sync.dma_start(out=vcat,
                          in_=D["vcat"].rearrange("(c p) d -> p c d", p=P))

        # ---------------- r1 / r64 build (transient pool) ----------------
        with tc.tile_pool(name="early", bufs=1) as early:
            r1int = early.tile([P, 16, T], BF, name="r1int")
            nc.sync.dma_start(out=r1int,
                              in_=D["r1in"].rearrange("c p t -> p c t"))
            tabr1 = early.tile([P, 4, T], BF, name="tabr1")
            nc.sync.dma_start(out=tabr1,
                              in_=D["tabr1"].rearrange("c p t -> p c t"))
            d64qt = early.tile([64, 16, T], BF, name="d64qt")
            nc.sync.dma_start(out=d64qt,
                              in_=D["d64q"].rearrange("p c t -> p c t"))
            d64kt = early.tile([64, 8, T], BF, name="d64kt")
            nc.sync.dma_start(out=d64kt,
                              in_=D["d64k"].rearrange("p c t -> p c t"))
            t64t = early.tile([64, 4, T], BF, name="t64t")
            nc.sync.dma_start(out=t64t,
                              in_=D["t64"].rearrange("p c t -> p c t"))
            u1 = early.tile([P, T], BF, name="u1")
            for g in range(4):
                # r1q[g] = qa1*c1q + qa1s*s1q ; r1k[g] = ka1*c1 + ka1s*s1
                pick.tt(u1, r1int[:, 4 + g, :], tabr1[:, 1, :], mult, T)
                pick.tt(r1q[:, g, :], r1int[:, g, :], tabr1[:, 0, :], mult, T)
                pick.tt(r1q[:, g, :], r1q[:, g, :], u1, add, T)
                pick.tt(u1, r1int[:, 12 + g, :], tabr1[:, 3, :], mult, T)
                pick.tt(r1k[:, g, :], r1int[:, 8 + g, :], tabr1[:, 2, :], mult, T)
                pick.tt(r1k[:, g, :], r1k[:, g, :], u1, add, T)
            u2 = early.tile([64, T], BF, name="u2")
            for j in range(8):
                pick.tt(u2, d64qt[:, 8 + j, :], t64t[:, 1, :], mult, T)
                pick.tt(r64q[:, j, :], d64qt[:, j, :], t64t[:, 0, :], mult, T)
                pick.tt(r64q[:, j, :], r64q[:, j, :], u2, add, T)
            for j in range(4):
                pick.tt(u2, d64kt[:, 4 + j, :], t64t[:, 3, :], mult, T)
                pick.tt(r64k[:, j, :], d64kt[:, j, :], t64t[:, 2, :], mult, T)
                pick.tt(r64k[:, j, :], r64k[:, j, :], u2, add, T)

        # ---------------- work pools ----------------
        mixp = ctx.enter_context(tc.tile_pool(name="mix", bufs=2))
        scr = ctx.enter_context(tc.tile_pool(name="scr", bufs=2))
        ptp = ctx.enter_context(tc.tile_pool(name="pt", bufs=2))
        tAp = ctx.enter_context(tc.tile_pool(name="tA", bufs=2))
        outp = ctx.enter_context(tc.tile_pool(name="out", bufs=2))
        spsum = ctx.enter_context(tc.tile_pool(name="sp", bufs=2, space="PSUM"))
        ypsum = ctx.enter_context(tc.tile_pool(name="yp", bufs=4, space="PSUM"))

        c2q, s2q = tabm[:, 0, :], tabm[:, 1, :]
        c2k, s2k = tabm[:, 2, :], tabm[:, 3, :]
        cBq, sBq = tabm[:, 4, :], tabm[:, 5, :]

        def mix_A(xt, g, cpos, spos, r1, tag):
            """[P,2,T] mix for config-A (d=256 rope + folded d128 part)."""
            qm = mixp.tile([P, 2, T], BF, tag=tag, name=tag)
            u = scr.tile([P, T], BF, tag="uA", name="uA")
            x0, x1 = xt[:, 2 * g, :], xt[:, 2 * g + 1, :]
            pick.tt(u, x1, spos, mult, T)
            pick.tt(qm[:, 0, :], x0, cpos, mult, T)
            pick.tt(qm[:, 0, :], qm[:, 0, :], u, sub, T)
            pick.tt(qm[:, 0, :], qm[:, 0, :], r1, add, T)
            pick.tt(u, x0, spos, mult, T)
            pick.tt(qm[:, 1, :], x1, cpos, mult, T)
            pick.tt(qm[:, 1, :], qm[:, 1, :], u, add, T)
            return qm

        def phase1(qm_chunks, km_chunks, pt):
            """QK + mask + exp for one head. qm/km: list of [P, T] APs."""
            ndc = len(qm_chunks)
            for c in range(TK):
                q0 = P * c
                sT = spsum.tile([P, T], F32, tag="sT", name="sT")
                pieces = ([(q0, 512), (512, T)] if c < 4 else [(q0, T)])
                for (a, b) in pieces:
                    for dc in range(ndc):
                        nc.tensor.matmul(sT[:, a:b],
                                         km_chunks[dc][:, q0:q0 + P],
                                         qm_chunks[dc][:, a:b],
                                         start=(dc == 0), stop=(dc == ndc - 1))
                nc.tensor.matmul(sT[:, q0:q0 + P], tri, iden,
                                 start=False, stop=True, skip_group_check=True)
                nc.scalar.activation(pt[:, c, q0:T], sT[:, q0:T], Exp)

        def phase2_A(pt, vm, tA):
            for qc in range(TK):
                y = ypsum.tile([P, 512], F32, tag="y", name="y")
                for c in range(qc + 1):
                    nc.tensor.matmul(y[:, 0:257],
                                     pt[:, c, P * qc:P * qc + P],
                                     vm[:, c, :],
                                     start=(c == 0), stop=(c == qc))
                pick.tsp(tA[:, qc, :], y[:, 0:256], y[:, 256:257], div,
                         256, psum=True)

        def phase2_B(pt, vm, tA, outt, hh):
            for qc in range(TK):
                y = ypsum.tile([P, 512], F32, tag="y", name="y")
                for c in range(qc + 1):
                    nc.tensor.matmul(y[:, 0:129],
                                     pt[:, c, P * qc:P * qc + P],
                                     vm[:, c, :],
                                     start=(c == 0), stop=(c == qc))
                pick.stt(outt[:, qc, 128 * hh:128 * hh + 128],
                         y[:, 0:128], y[:, 128:129],
                         tA[:, qc, 128 * hh:128 * hh + 128],
                         div, add, 128, psum=True)

        state = {}

        def do_A(g):
            qm = mix_A(qa2t, g, c2q, s2q, r1q[:, g, :], "qmA")
            ka2c = mixp.tile([P, 2, T], BF, tag="ka2c", name="ka2c")
            nc.sync.dma_start(out=ka2c, in_=D["ka2"].rearrange(
                "c p t -> p c t")[:, 2 * g:2 * g + 2, :])
            km = mix_A(ka2c, 0, c2k, s2k, r1k[:, g, :], "kmA")
            vm = mixp.tile([P, TK, 257], BF, tag="vmA", name="vmA")
            uv = scr.tile([P, TK, P], BF, tag="uvA", name="uvA")
            pick.tsp(vm[:, :, 0:256], vcat[:, :, 512 + 256 * g:768 + 256 * g],
                     wv[:, 1:2], mult, 2048)
            pick.tsp(uv, vcat[:, :, 128 * g:128 * g + P], wv[:, 0:1],
                     mult, 1024)
            pick.tt(vm[:, :, 0:P], vm[:, :, 0:P], uv, add, 1024)
            nc.vector.memset(vm[:, :, 256:257], 1.0)
            pt = ptp.tile([P, TK, T], BF, tag="pt", name="ptA")
            phase1([qm[:, 0, :], qm[:, 1, :]], [km[:, 0, :], km[:, 1, :]], pt)
            tA = tAp.tile([P, TK, 256], BF, tag="tA", name="tA")
            phase2_A(pt, vm, tA)
            state[g] = tA

        def do_B(h):
            g, hh = h // 2, h % 2
            if hh == 0:
                km = mixp.tile([P, T], BF, tag="kmB", name="kmB")
                pick.tt(km[0:64, :], r1k[0:64, g, :], r64k[:, g, :], add, T)
                pick.cp(km[64:P, :], r1k[64:P, g, :], T)
                vm = mixp.tile([P, TK, 129], BF, tag="vmB", name="vmB")
                uv = scr.tile([P, TK, 64], BF, tag="uvB", name="uvB")
                pick.tsp(vm[:, :, 0:128], vcat[:, :, 128 * g:128 * g + P],
                         wv[:, 3:4], mult, 1024)
                pick.tsp(uv, vcat[:, :, 1536 + 64 * g:1600 + 64 * g],
                         wv[:, 2:3], mult, 512)
                pick.tt(vm[:, :, 0:64], vm[:, :, 0:64], uv, add, 512)
                nc.vector.memset(vm[:, :, 128:129], 1.0)
                state[("B", g)] = (km, vm)
                outt = outp.tile([P, TK, 256], BF, tag="outt", name="outt")
                state[("o", g)] = outt
            km, vm = state[("B", g)]
            outt = state[("o", g)]
            qm = mixp.tile([P, T], BF, tag="qmB", name="qmB")
            u = scr.tile([P, T], BF, tag="uB", name="uB")
            sg = scr.tile([P, T], BF, tag="sgB", name="sgB")
            pick.cp(sg[0:64, :], qa2t[64:P, h, :], T)
            pick.cp(sg[64:P, :], qa2t[0:64, h, :], T)
            pick.tt(u, sg, sBq, mult, T)
            pick.tt(qm, qa2t[:, h, :], cBq, mult, T)
            pick.tt(qm, qm, u, add, T)
            pick.tt(qm[0:64, :], qm[0:64, :], r64q[:, h, :], add, T)
            pt = ptp.tile([P, TK, T], BF, tag="pt", name="ptB")
            phase1([qm], [km], pt)
            phase2_B(pt, vm, state[g], outt, hh)
            if hh == 1:
                nc.sync.dma_start(
                    out=outQ.rearrange("(c p) d -> p c d", p=P)
                    [:, :, 256 * g:256 * g + 256],
                    in_=outt)

        for g in range(4):
            do_A(g)
            do_B(2 * g)
            do_B(2 * g + 1)

    nc.compile()
    return nc


# ---------------------------------------------------------------------------
# Host side
# ---------------------------------------------------------------------------

def _rope_tabs(pos, d, scale=1.0):
    """cos/sin tables [d, T]; sin SIGNED math-order (rows < d/2 negated)."""
    inv = 1.0 / (10000.0 ** (np.arange(0, d, 2, dtype=np.float32) / d))
    ang = inv[:, None] * pos[None, :].astype(np.float32)
    ang = np.concatenate([ang, ang], 0)
    c = (scale * np.cos(ang)).astype(np.float32)
    s = (scale * np.sin(ang)).astype(np.float32)
    s[: d // 2] *= -1.0
    return c, s


def _sigma(x, half):
    sh = x.shape
    y = x.reshape(-1, 2, half, *sh[1:])
    return np.ascontiguousarray(y[:, ::-1].reshape(sh))


def make_core_inputs(q, k, v, pos, weights, s, cfg: KCfg = FULL):
    """q,k,v: [T, 2048] fp32 for one batch; returns per-core input dict."""
    bf = lambda x: np.ascontiguousarray(x, dtype=NPBF)
    w0, w1, w2, w3 = [float(x) for x in weights]
    fA2 = 1.0 / 16.0
    fB2 = 1.0 / math.sqrt(128.0)

    qa1 = q[:, 512 * s:512 * s + 512].T          # [512, T]
    qa2 = q[:, 1024 * s:1024 * s + 1024].T       # [1024, T]
    ka1 = k[:, 512 * s:512 * s + 512].T
    ka2 = k[:, 1024 * s:1024 * s + 1024].T
    kb1 = k[:, 256 * s:256 * s + 256].T          # [256, T]

    r1in = np.concatenate([
        qa1.reshape(4, P, T), _sigma(qa1.reshape(4, P, T).reshape(512, T), 64)
        .reshape(4, P, T),
        ka1.reshape(4, P, T), _sigma(ka1, 64).reshape(4, P, T)], 0)
    # NOTE: sigma must be applied per 128-row block; reshape(4,P,T) blocks are
    # 128 rows so _sigma on the flat [512,T] with half=64 handles all blocks.

    c1q, s1q = _rope_tabs(pos, 128, fA2 * w0 * w0)
    c1, s1 = _rope_tabs(pos, 128)
    tabr1 = np.stack([c1q, s1q, c1, s1])

    d64 = qa1.reshape(8, 64, T)                  # ch j = head j's d64 slice
    d64q = np.concatenate([d64, _sigma(qa1.reshape(512, T), 32).reshape(8, 64, T)], 0)
    d64q = np.ascontiguousarray(d64q.transpose(1, 0, 2))      # [64, 16, T]
    d64kk = kb1.reshape(4, 64, T)
    d64k = np.concatenate([d64kk, _sigma(kb1, 32).reshape(4, 64, T)], 0)
    d64k = np.ascontiguousarray(d64k.transpose(1, 0, 2))      # [64, 8, T]

    c64q, s64q = _rope_tabs(pos, 64, fB2 * w3 * w2)
    c64k, s64k = _rope_tabs(pos, 64, w2 / w3)
    t64 = np.stack([c64q, s64q, c64k, s64k])
    t64 = np.ascontiguousarray(t64.transpose(1, 0, 2))        # [64, 4, T]

    c2q, s2q = _rope_tabs(pos, 256, fA2 * w0 * w1)
    c2k, s2k = _rope_tabs(pos, 256, w1 / w0)
    cBq, sBq = _rope_tabs(pos, 128, fB2 * w3 * w3)
    # positive-half tables for d256 (chunk ops use explicit sub/add)
    tabm = np.stack([c2q[:P], -s2q[:P], c2k[:P], -s2k[:P], cBq, sBq])
    # s2*[:P] is the NEGATED first half (signed); chunk0 op subtracts, so
    # store +sin = -(signed first half); chunk1 needs +sin too.

    vcat = np.concatenate([v[:, 512 * s:512 * s + 512],
                           v[:, 1024 * s:1024 * s + 1024],
                           v[:, 256 * s:256 * s + 256]], 1)   # [T, 1792]

    tri = np.zeros((P, P), np.float32)
    j, kk = np.mgrid[0:P, 0:P]
    tri[j < kk] = NEG
    consts = np.stack([tri, np.eye(P, dtype=np.float32)])

    arrs = {
        "r1in": bf(r1in), "tabr1": bf(tabr1), "d64q": bf(d64q),
        "d64k": bf(d64k), "t64": bf(t64),
        "qa2": bf(qa2.reshape(8, P, T)),
        "ka2": bf(ka2.reshape(8, P, T)), "tabm": bf(tabm),
        "vcat": bf(vcat), "consts": bf(consts),
        "wvec": np.tile(np.asarray(weights, np.float32)[None, :], (P, 1)),
    }
    return arrs


_PROGRAM_CACHE = {}
TRACE = False
LAST_RESULT = None


def kernel(q_m, k_m, v_m, weights, attention_mask, position_ids):
    global LAST_RESULT
    from concourse.bass_utils import run_bass_kernel_spmd

    cfg = FULL
    q_m = np.asarray(q_m, np.float32)
    k_m = np.asarray(k_m, np.float32)
    v_m = np.asarray(v_m, np.float32)
    weights = np.asarray(weights, np.float32)
    attention_mask = np.asarray(attention_mask, np.float32)
    position_ids = np.asarray(position_ids)
    B, Tq, H = q_m.shape

    causal = np.where(np.tril(np.ones((Tq, Tq), bool)), 0.0, NEG).astype(np.float32)
    for b in range(B):
        assert np.array_equal(attention_mask[b, 0], causal), "non-causal mask"

    if "nc" not in _PROGRAM_CACHE:
        _PROGRAM_CACHE["nc"] = build_program(cfg)
    nc = _PROGRAM_CACHE["nc"]

    in_maps = []
    for b in range(B):
        for s in range(2):
            in_maps.append(make_core_inputs(
                q_m[b], k_m[b], v_m[b], position_ids[b], weights, s, cfg))
    res = run_bass_kernel_spmd(nc, in_maps, list(range(8)), trace=TRACE)
    LAST_RESULT = res
    out = np.zeros((B, Tq, H), np.float32)
    for b in range(B):
        for s in range(2):
            out[b, :, 1024 * s:1024 * s + 1024] = \
                res.results[2 * b + s]["outQ"].astype(np.float32)
    return out


# revision 8
# speedup vs baseline: 1.6941x; 1.3666x over previous
"""Trainium2 Bass kernel for nn_MixedAttnHeadEmbed (mixed-head-config attention).

v2: bf16 end-to-end with [q,d]-layout outputs.

Math (per batch b): two attention configs share q_m/k_m/v_m [B,T,2048]:
  A: h=8  heads, d_max=256, mixing e in {1024,2048} -> d in {128,256}, w0,w1
  B: h=16 heads, d_max=128, mixing e in {1024,2048} -> d in {64,128},  w2,w3
Sharding: 8 cores = 4 batches x 2 shards; shard s owns A-heads [4s,4s+4) and
B-heads [8s,8s+8) -> output cols [1024s, 1024s+1024), written [T, 1024].

Device design notes:
 - Everything bf16 on SBUF (DVE 2x/4x fast modes, full-rate PE, half DMA);
   PSUM f32 only for matmul accumulation.
 - Weight folding moves all mix scalars into host-precomputed rope tables:
     qmA = fA^2 w0 (w0 P(r1q) + w1 R2q),   kmA = P(r1k) + (w1/w0) R2k
     qmB = fB^2 w3 (w3 R128(x) + w2 P(r64q)), kmB = r1k[g] + (w2/w3) P(r64k)
   where r1 = unscaled rope128 (k-side shared between A and B).
 - Rope rotations are free: sigma-permuted DATA copies ship from host, so
   rope(x) = x*c + xs*s with signed math-order sin tables (3 TT per tile).
 - Scores computed transposed sT[k,q] per k-chunk; causal mask applied ON THE
   PE (strict-tri(-1e9) @ I accumulated into the diagonal block); exp on ACT
   (one inst per chunk) into an SBUF bf16 pt [128, 8, T].
 - Phase 2 emits y in [q-part, d-free] via pt-stationary matmuls with an
   AUGMENTED V (ones column) so the softmax denominator is the last y column;
   normalize = tensor_scalar divide by that per-partition column (no
   reciprocal, no transposes, output lands in natural [T, H] layout).
 - A static engine balancer routes elementwise work DVE/Pool by modeled cost.
"""

import math
from contextlib import ExitStack
from dataclasses import dataclass

import numpy as np
import ml_dtypes

import concourse.bass as bass
import concourse.mybir as mybir
import concourse.tile as tile
from concourse import bacc

F32 = mybir.dt.float32
BF = mybir.dt.bfloat16
NPBF = ml_dtypes.bfloat16
NEG = -1e9
P = 128
T = 1024
TK = T // P


@dataclass(frozen=True)
class KCfg:
    pass


FULL = KCfg()

mult = mybir.AluOpType.mult
add = mybir.AluOpType.add
sub = mybir.AluOpType.subtract
div = mybir.AluOpType.divide
Exp = mybir.ActivationFunctionType.Exp


class _Pick:
    """Cost-model-accurate static balancer for DVE / Pool elementwise ops."""

    def __init__(self, nc):
        self.nc = nc
        self.load = {"dve": 0.0, "pool": 0.0}

    def _choose(self, cd, cp):
        if self.load["dve"] + cd <= self.load["pool"] + cp:
            self.load["dve"] += cd
            return self.nc.vector
        self.load["pool"] += cp
        return self.nc.gpsimd

    def tt(self, out, a, b, op, free, psum=False):
        cd = (1.0417 * free + 125) if psum else (0.52 * free + 60)
        eng = self._choose(cd, 0.8333 * free + 120)
        eng.tensor_tensor(out, a, b, op)

    def tsp(self, out, a, scal, op, free, psum=False):
        cd = (1.0417 * free + 125) if psum else (0.26 * free + 60)
        eng = self._choose(cd, 0.8333 * free + 120)
        eng.tensor_scalar(out=out, in0=a, scalar1=scal, scalar2=None, op0=op)

    def stt(self, out, a, scal, b, op0, op1, free, psum=False):
        cd = (1.0417 * free + 125) if psum else (1.0417 * free + 60)
        eng = self._choose(cd, 0.8333 * free + 120)
        eng.scalar_tensor_tensor(out=out, in0=a, scalar=scal, in1=b,
                                 op0=op0, op1=op1)

    def cp(self, dst, src, free, psum=False):
        cd = (1.0417 * free + 125) if psum else (0.26 * free + 60)
        eng = self._choose(cd, 0.8333 * free + 120)
        eng.tensor_copy(dst, src)


def build_program(cfg: KCfg = FULL):
    nc = bacc.Bacc("TRN2", target_bir_lowering=False)

    def dram(name, shape, dt=BF, out=False):
        return nc.declare_dram_parameter(name, list(shape), dt, isOutput=out)

    D = {
        # g-major: ch 4g+(qa1, qa1s, ka1, ka1s)
        "r1in": dram("r1in", (16, P, T)),
        "tabr1": dram("tabr1", (4, P, T)),     # c1q s1q c1 s1 (signed)
        # packed half-partition: rows 0:64 sigma32 data, rows 64:128 raw data
        "d64q": dram("d64q", (8, P, T)),       # ch j = B-head j d64 slice
        "d64k": dram("d64k", (4, P, T)),       # ch g = B-kv g d64 slice
        "t64": dram("t64", (4, P, T)),         # c64q s64q c64k s64k (dup halves)
        "qa2": dram("qa2", (8, P, T)),
        "ka2": dram("ka2", (8, P, T)),
        "tabm": dram("tabm", (6, P, T)),       # c2q s2q c2k s2k cBq sBq
        "vcat": dram("vcat", (T, 1792)),       # va1 512 | va2 1024 | vb1 256
        "consts": dram("consts", (2, P, P)),   # tri(NEG strict upper), iden
        "wvec": dram("wvec", (P, 4), dt=F32),
    }
    outQ = dram("outQ", (T, 1024), out=True)
    r1in_r = D["r1in"].rearrange("c p t -> p c t")
    qa2_r = D["qa2"].rearrange("c p t -> p c t")
    ka2_r = D["ka2"].rearrange("c p t -> p c t")
    d64q_r = D["d64q"].rearrange("c p t -> p c t")
    d64k_r = D["d64k"].rearrange("c p t -> p c t")
    vcat_r = D["vcat"].rearrange("(c p) d -> p c d", p=P)

    with ExitStack() as ctx:
        tc = ctx.enter_context(tile.TileContext(nc))
        pers = ctx.enter_context(tc.tile_pool(name="pers", bufs=1))
        pick = _Pick(nc)

        # ---------------- persistent tiles ----------------
        r1q = pers.tile([P, 4, T], BF, name="r1q")
        r1k = pers.tile([P, 4, T], BF, name="r1k")
        tabm = pers.tile([P, 6, T], BF, name="tabm")
        cst = pers.tile([P, 2, P], BF, name="cst")
        wv = pers.tile([P, 4], F32, name="wv")
        qa2t = pers.tile([P, 8, T], BF, name="qa2t")
        t64t = pers.tile([P, 4, T], BF, name="t64t")
        d64qt = pers.tile([P, 8, T], BF, name="d64qt")
        d64kt = pers.tile([P, 4, T], BF, name="d64kt")
        tri, iden = cst[:, 0, :], cst[:, 1, :]

        # ---------------- r1 build (transient pool), consumption-ordered DMA
        with tc.tile_pool(name="early", bufs=1) as early:
            tabr1 = early.tile([P, 4, T], BF, name="tabr1")
            nc.sync.dma_start(out=tabr1,
                              in_=D["tabr1"].rearrange("c p t -> p c t"))
            r1int = early.tile([P, 16, T], BF, name="r1int")
            nc.sync.dma_start(out=r1int[:, 0:4, :], in_=r1in_r[:, 0:4, :])
            nc.sync.dma_start(out=tabm, in_=D["tabm"].rearrange("c p t -> p c t"))
            nc.sync.dma_start(out=qa2t[:, 0:2, :], in_=qa2_r[:, 0:2, :])
            nc.sync.dma_start(out=cst, in_=D["consts"].rearrange("c p t -> p c t"))
            nc.sync.dma_start(out=wv, in_=D["wvec"][:, :])
            nc.sync.dma_start(out=t64t, in_=D["t64"].rearrange("c p t -> p c t"))
            nc.sync.dma_start(out=d64qt[:, 0:2, :], in_=d64q_r[:, 0:2, :])
            nc.sync.dma_start(out=d64kt[:, 0:1, :], in_=d64k_r[:, 0:1, :])
            u1 = early.tile([P, T], BF, name="u1")

            def r1build(g):
                pick.tt(u1, r1int[:, 4 * g + 1, :], tabr1[:, 1, :], mult, T)
                pick.tt(r1q[:, g, :], r1int[:, 4 * g, :], tabr1[:, 0, :], mult, T)
                pick.tt(r1q[:, g, :], r1q[:, g, :], u1, add, T)
                pick.tt(u1, r1int[:, 4 * g + 3, :], tabr1[:, 3, :], mult, T)
                pick.tt(r1k[:, g, :], r1int[:, 4 * g + 2, :], tabr1[:, 2, :], mult, T)
                pick.tt(r1k[:, g, :], r1k[:, g, :], u1, add, T)

            r1build(0)
            nc.scalar.dma_start(out=r1int[:, 4:16, :], in_=r1in_r[:, 4:16, :])
            for g in range(1, 4):
                r1build(g)

        # ---------------- work pools ----------------
        mixp = ctx.enter_context(tc.tile_pool(name="mix", bufs=2))
        scr = ctx.enter_context(tc.tile_pool(name="scr", bufs=1))
        ptp = ctx.enter_context(tc.tile_pool(name="pt", bufs=2))
        tAp = ctx.enter_context(tc.tile_pool(name="tA", bufs=2))
        outp = ctx.enter_context(tc.tile_pool(name="out", bufs=2))
        spsum = ctx.enter_context(tc.tile_pool(name="sp", bufs=2, space="PSUM"))
        ypsum = ctx.enter_context(tc.tile_pool(name="yp", bufs=4, space="PSUM"))

        # late bulk loads (scalar queue: off the group-0 critical path)
        nc.scalar.dma_start(out=qa2t[:, 2:8, :], in_=qa2_r[:, 2:8, :])
        nc.scalar.dma_start(out=d64qt[:, 2:8, :], in_=d64q_r[:, 2:8, :])
        nc.scalar.dma_start(out=d64kt[:, 1:4, :], in_=d64k_r[:, 1:4, :])

        c2q, s2q = tabm[:, 0, :], tabm[:, 1, :]
        c2k, s2k = tabm[:, 2, :], tabm[:, 3, :]
        cBq, sBq = tabm[:, 4, :], tabm[:, 5, :]

        state = {}

        def prefetch(g):
            if g >= 4 or ("ka2c", g) in state:
                return
            ka2c = mixp.tile([P, 2, T], BF, tag="ka2c", name="ka2c")
            nc.gpsimd.dma_start(out=ka2c, in_=ka2_r[:, 2 * g:2 * g + 2, :])
            va2g = mixp.tile([P, TK, 256], BF, tag="va2g", name="va2g")
            nc.gpsimd.dma_start(out=va2g, in_=vcat_r[:, :, 512 + 256 * g:768 + 256 * g])
            va1g = mixp.tile([P, TK, P], BF, tag="va1g", name="va1g")
            nc.gpsimd.dma_start(out=va1g, in_=vcat_r[:, :, P * g:P * g + P])
            vb1g = mixp.tile([P, TK, 64], BF, tag="vb1g", name="vb1g")
            nc.gpsimd.dma_start(out=vb1g, in_=vcat_r[:, :, 1536 + 64 * g:1600 + 64 * g])
            state[("ka2c", g)] = ka2c
            state[("va2", g)] = va2g
            state[("va1", g)] = va1g
            state[("vb1", g)] = vb1g

        def mix_A(xt, ch, cpos, spos, r1, tag):
            """[P,2,T] mix for config-A (d=256 rope + folded d128 part)."""
            qm = mixp.tile([P, 2, T], BF, tag=tag, name=tag)
            u = scr.tile([P, T], BF, tag="uA", name="uA")
            x0, x1 = xt[:, ch, :], xt[:, ch + 1, :]
            pick.tt(u, x1, spos, mult, T)
            pick.tt(qm[:, 0, :], x0, cpos, mult, T)
            pick.tt(qm[:, 0, :], qm[:, 0, :], u, sub, T)
            pick.tt(qm[:, 0, :], qm[:, 0, :], r1, add, T)
            pick.tt(u, x0, spos, mult, T)
            pick.tt(qm[:, 1, :], x1, cpos, mult, T)
            pick.tt(qm[:, 1, :], qm[:, 1, :], u, add, T)
            return qm

        def r64build(src, ch, ctab, stab, dst, dch):
            """dst[0:64, dch] = rope64 of packed src channel ch."""
            u = scr.tile([P, T], BF, tag="u64", name="u64")
            pick.tt(dst[0:64, dch, :], src[0:64, ch, :], stab[0:64, :], mult, T)
            pick.tt(u[64:P, :], src[64:P, ch, :], ctab[64:P, :], mult, T)
            pick.cp(u[0:64, :], u[64:P, :], T)
            pick.tt(dst[0:64, dch, :], dst[0:64, dch, :], u[0:64, :], add, T)

        def phase1(qm_chunks, km_chunks, pt):
            ndc = len(qm_chunks)
            for c in range(TK):
                q0 = P * c
                sT = spsum.tile([P, T], F32, tag="sT", name="sT")
                pieces = ([(q0, 512), (512, T)] if c < 4 else [(q0, T)])
                for (a, b) in pieces:
                    for dc in range(ndc):
                        nc.tensor.matmul(sT[:, a:b],
                                         km_chunks[dc][:, q0:q0 + P],
                                         qm_chunks[dc][:, a:b],
                                         start=(dc == 0), stop=(dc == ndc - 1))
                nc.tensor.matmul(sT[:, q0:q0 + P], tri, iden,
                                 start=False, stop=True, skip_group_check=True)
                nc.scalar.activation(pt[:, c, q0:T], sT[:, q0:T], Exp)

        def phase2_A(pt, vm, tA):
            for qc in range(TK):
                y = ypsum.tile([P, 512], F32, tag="y", name="y")
                for c in range(qc + 1):
                    nc.tensor.matmul(y[:, 0:257],
                                     pt[:, c, P * qc:P * qc + P],
                                     vm[:, c, :],
                                     start=(c == 0), stop=(c == qc))
                pick.tsp(tA[:, qc, :], y[:, 0:256], y[:, 256:257], div,
                         256, psum=True)

        def phase2_B(pt, vm, tA, outt, hh):
            for qc in range(TK):
                y = ypsum.tile([P, 512], F32, tag="y", name="y")
                for c in range(qc + 1):
                    nc.tensor.matmul(y[:, 0:129],
                                     pt[:, c, P * qc:P * qc + P],
                                     vm[:, c, :],
                                     start=(c == 0), stop=(c == qc))
                pick.stt(outt[:, qc, 128 * hh:128 * hh + 128],
                         y[:, 0:128], y[:, 128:129],
                         tA[:, qc, 128 * hh:128 * hh + 128],
                         div, add, 128, psum=True)

        def do_A(g):
            prefetch(g)
            prefetch(g + 1)
            qm = mix_A(qa2t, 2 * g, c2q, s2q, r1q[:, g, :], "qmA")
            km = mix_A(state[("ka2c", g)], 0, c2k, s2k, r1k[:, g, :], "kmA")
            vm = mixp.tile([P, TK, 257], BF, tag="vmA", name="vmA")
            uv = scr.tile([P, TK, P], BF, tag="uvA", name="uvA")
            pick.tsp(vm[:, :, 0:256], state[("va2", g)], wv[:, 1:2], mult, 2048)
            pick.tsp(uv, state[("va1", g)], wv[:, 0:1], mult, 1024)
            pick.tt(vm[:, :, 0:P], vm[:, :, 0:P], uv, add, 1024)
            nc.vector.memset(vm[:, :, 256:257], 1.0)
            pt = ptp.tile([P, TK, T], BF, tag="pt", name="ptA")
            phase1([qm[:, 0, :], qm[:, 1, :]], [km[:, 0, :], km[:, 1, :]], pt)
            tA = tAp.tile([P, TK, 256], BF, tag="tA", name="tA")
            phase2_A(pt, vm, tA)
            state[g] = tA

        def do_B(h):
            g, hh = h // 2, h % 2
            if hh == 0:
                r64g = mixp.tile([64, 2, T], BF, tag="r64q", name="r64q")
                r64build(d64qt, 2 * g, t64t[:, 0, :], t64t[:, 1, :], r64g, 0)
                r64build(d64qt, 2 * g + 1, t64t[:, 0, :], t64t[:, 1, :], r64g, 1)
                r64kg = mixp.tile([64, 1, T], BF, tag="r64k", name="r64k")
                r64build(d64kt, g, t64t[:, 2, :], t64t[:, 3, :], r64kg, 0)
                km = mixp.tile([P, T], BF, tag="kmB", name="kmB")
                pick.tt(km[0:64, :], r1k[0:64, g, :], r64kg[:, 0, :], add, T)
                pick.cp(km[64:P, :], r1k[64:P, g, :], T)
                vm = mixp.tile([P, TK, 129], BF, tag="vmB", name="vmB")
                uv = scr.tile([P, TK, 64], BF, tag="uvB", name="uvB")
                pick.tsp(vm[:, :, 0:128], state[("va1", g)], wv[:, 3:4], mult, 1024)
                pick.tsp(uv, state[("vb1", g)], wv[:, 2:3], mult, 512)
                pick.tt(vm[:, :, 0:64], vm[:, :, 0:64], uv, add, 512)
                nc.vector.memset(vm[:, :, 128:129], 1.0)
                state[("B", g)] = (km, vm, r64g)
                outt = outp.tile([P, TK, 256], BF, tag="outt", name="outt")
                state[("o", g)] = outt
            km, vm, r64g = state[("B", g)]
            outt = state[("o", g)]
            qm = mixp.tile([P, T], BF, tag="qmB", name="qmB")
            u = scr.tile([P, T], BF, tag="uB", name="uB")
            sg = scr.tile([P, T], BF, tag="sgB", name="sgB")
            pick.cp(sg[0:64, :], qa2t[64:P, h, :], T)
            pick.cp(sg[64:P, :], qa2t[0:64, h, :], T)
            pick.tt(u, sg, sBq, mult, T)
            pick.tt(qm, qa2t[:, h, :], cBq, mult, T)
            pick.tt(qm, qm, u, add, T)
            pick.tt(qm[0:64, :], qm[0:64, :], r64g[:, hh, :], add, T)
            pt = ptp.tile([P, TK, T], BF, tag="pt", name="ptB")
            phase1([qm], [km], pt)
            phase2_B(pt, vm, state[g], outt, hh)
            if hh == 1:
                nc.sync.dma_start(
                    out=outQ.rearrange("(c p) d -> p c d", p=P)
                    [:, :, 256 * g:256 * g + 256],
                    in_=outt)

        for g in range(4):
            do_A(g)
            do_B(2 * g)
            do_B(2 * g + 1)

    nc.compile()
    return nc


# ---------------------------------------------------------------------------
# Host side
# ---------------------------------------------------------------------------

def _rope_tabs(pos, d, scale=1.0):
    """cos/sin tables [d, T]; sin SIGNED math-order (rows < d/2 negated)."""
    inv = 1.0 / (10000.0 ** (np.arange(0, d, 2, dtype=np.float32) / d))
    ang = inv[:, None] * pos[None, :].astype(np.float32)
    ang = np.concatenate([ang, ang], 0)
    c = (scale * np.cos(ang)).astype(np.float32)
    s = (scale * np.sin(ang)).astype(np.float32)
    s[: d // 2] *= -1.0
    return c, s


def _sigma(x, half):
    sh = x.shape
    y = x.reshape(-1, 2, half, *sh[1:])
    return np.ascontiguousarray(y[:, ::-1].reshape(sh))


def make_core_inputs(q, k, v, pos, weights, s, cfg: KCfg = FULL):
    """q,k,v: [T, 2048] fp32 for one batch; returns per-core input dict."""
    bf = lambda x: np.ascontiguousarray(x, dtype=NPBF)
    w0, w1, w2, w3 = [float(x) for x in weights]
    fA2 = 1.0 / 16.0
    fB2 = 1.0 / math.sqrt(128.0)

    qa1 = q[:, 512 * s:512 * s + 512].T          # [512, T]
    qa2 = q[:, 1024 * s:1024 * s + 1024].T       # [1024, T]
    ka1 = k[:, 512 * s:512 * s + 512].T
    ka2 = k[:, 1024 * s:1024 * s + 1024].T
    kb1 = k[:, 256 * s:256 * s + 256].T          # [256, T]

    qa1b = qa1.reshape(4, P, T)
    qa1s = _sigma(qa1, 64).reshape(4, P, T)
    ka1b = ka1.reshape(4, P, T)
    ka1s = _sigma(ka1, 64).reshape(4, P, T)
    r1in = np.stack([qa1b, qa1s, ka1b, ka1s], 1).reshape(16, P, T)  # g-major

    c1q, s1q = _rope_tabs(pos, 128, fA2 * w0 * w0)
    c1, s1 = _rope_tabs(pos, 128)
    tabr1 = np.stack([c1q, s1q, c1, s1])

    # packed d64: rows 0:64 sigma32 data, rows 64:128 raw data
    dq = qa1.reshape(8, 64, T)
    dqs = _sigma(qa1, 32).reshape(8, 64, T)
    d64q = np.concatenate([dqs, dq], 1)                       # [8, 128, T]
    dk = kb1.reshape(4, 64, T)
    dks = _sigma(kb1, 32).reshape(4, 64, T)
    d64k = np.concatenate([dks, dk], 1)                       # [4, 128, T]

    c64q, s64q = _rope_tabs(pos, 64, fB2 * w3 * w2)
    c64k, s64k = _rope_tabs(pos, 64, w2 / w3)
    t64 = np.stack([np.concatenate([c64q, c64q], 0),
                    np.concatenate([s64q, s64q], 0),
                    np.concatenate([c64k, c64k], 0),
                    np.concatenate([s64k, s64k], 0)])         # [4, 128, T]

    c2q, s2q = _rope_tabs(pos, 256, fA2 * w0 * w1)
    c2k, s2k = _rope_tabs(pos, 256, w1 / w0)
    cBq, sBq = _rope_tabs(pos, 128, fB2 * w3 * w3)
    tabm = np.stack([c2q[:P], -s2q[:P], c2k[:P], -s2k[:P], cBq, sBq])

    vcat = np.concatenate([v[:, 512 * s:512 * s + 512],
                           v[:, 1024 * s:1024 * s + 1024],
                           v[:, 256 * s:256 * s + 256]], 1)   # [T, 1792]

    tri = np.zeros((P, P), np.float32)
    j, kk = np.mgrid[0:P, 0:P]
    tri[j < kk] = NEG
    consts = np.stack([tri, np.eye(P, dtype=np.float32)])

    arrs = {
        "r1in": bf(r1in), "tabr1": bf(tabr1), "d64q": bf(d64q),
        "d64k": bf(d64k), "t64": bf(t64),
        "qa2": bf(qa2.reshape(8, P, T)),
        "ka2": bf(ka2.reshape(8, P, T)), "tabm": bf(tabm),
        "vcat": bf(vcat), "consts": bf(consts),
        "wvec": np.tile(np.asarray(weights, np.float32)[None, :], (P, 1)),
    }
    del d64q, d64k  # noqa
    return arrs


_PROGRAM_CACHE = {}
TRACE = False
LAST_RESULT = None


def kernel(q_m, k_m, v_m, weights, attention_mask, position_ids):
    global LAST_RESULT
    from concourse.bass_utils import run_bass_kernel_spmd

    cfg = FULL
    q_m = np.asarray(q_m, np.float32)
    k_m = np.asarray(k_m, np.float32)
    v_m = np.asarray(v_m, np.float32)
    weights = np.asarray(weights, np.float32)
    attention_mask = np.asarray(attention_mask, np.float32)
    position_ids = np.asarray(position_ids)
    B, Tq, H = q_m.shape

    causal = np.where(np.tril(np.ones((Tq, Tq), bool)), 0.0, NEG).astype(np.float32)
    for b in range(B):
        assert np.array_equal(attention_mask[b, 0], causal), "non-causal mask"

    if "nc" not in _PROGRAM_CACHE:
        _PROGRAM_CACHE["nc"] = build_program(cfg)
    nc = _PROGRAM_CACHE["nc"]

    in_maps = []
    for b in range(B):
        for s in range(2):
            in_maps.append(make_core_inputs(
                q_m[b], k_m[b], v_m[b], position_ids[b], weights, s, cfg))
    res = run_bass_kernel_spmd(nc, in_maps, list(range(8)), trace=TRACE)
    LAST_RESULT = res
    out = np.zeros((B, Tq, H), np.float32)
    for b in range(B):
        for s in range(2):
            out[b, :, 1024 * s:1024 * s + 1024] = \
                res.results[2 * b + s]["outQ"].astype(np.float32)
    return out


# revision 14
# speedup vs baseline: 1.9169x; 1.1315x over previous
"""Trainium2 Bass kernel for nn_MixedAttnHeadEmbed (mixed-head-config attention).

v2: bf16 end-to-end with [q,d]-layout outputs.

Math (per batch b): two attention configs share q_m/k_m/v_m [B,T,2048]:
  A: h=8  heads, d_max=256, mixing e in {1024,2048} -> d in {128,256}, w0,w1
  B: h=16 heads, d_max=128, mixing e in {1024,2048} -> d in {64,128},  w2,w3
Sharding: 8 cores = 4 batches x 2 shards; shard s owns A-heads [4s,4s+4) and
B-heads [8s,8s+8) -> output cols [1024s, 1024s+1024), written [T, 1024].

Device design notes:
 - Everything bf16 on SBUF (DVE 2x/4x fast modes, full-rate PE, half DMA);
   PSUM f32 only for matmul accumulation.
 - Weight folding moves all mix scalars into host-precomputed rope tables:
     qmA = fA^2 w0 (w0 P(r1q) + w1 R2q),   kmA = P(r1k) + (w1/w0) R2k
     qmB = fB^2 w3 (w3 R128(x) + w2 P(r64q)), kmB = r1k[g] + (w2/w3) P(r64k)
   where r1 = unscaled rope128 (k-side shared between A and B).
 - Rope rotations are free: sigma-permuted DATA copies ship from host, so
   rope(x) = x*c + xs*s with signed math-order sin tables (3 TT per tile).
 - Scores computed transposed sT[k,q] per k-chunk; causal mask applied ON THE
   PE (strict-tri(-1e9) @ I accumulated into the diagonal block); exp on ACT
   (one inst per chunk) into an SBUF bf16 pt [128, 8, T].
 - Phase 2 emits y in [q-part, d-free] via pt-stationary matmuls with an
   AUGMENTED V (ones column) so the softmax denominator is the last y column;
   normalize = tensor_scalar divide by that per-partition column (no
   reciprocal, no transposes, output lands in natural [T, H] layout).
 - A static engine balancer routes elementwise work DVE/Pool by modeled cost.
"""

import math
from contextlib import ExitStack
from dataclasses import dataclass

import numpy as np
import ml_dtypes

import concourse.bass as bass
import concourse.mybir as mybir
import concourse.tile as tile
from concourse import bacc

F32 = mybir.dt.float32
BF = mybir.dt.bfloat16
NPBF = ml_dtypes.bfloat16
NEG = -1e9
P = 128
T = 1024
TK = T // P


@dataclass(frozen=True)
class KCfg:
    pass


FULL = KCfg()

mult = mybir.AluOpType.mult
add = mybir.AluOpType.add
sub = mybir.AluOpType.subtract
div = mybir.AluOpType.divide
Exp = mybir.ActivationFunctionType.Exp


class _Pick:
    """Cost-model-accurate static balancer for DVE / Pool elementwise ops."""

    def __init__(self, nc):
        self.nc = nc
        self.load = {"dve": 0.0, "pool": 0.0}

    def _choose(self, cd, cp, psum=False):
        # GPSIMD cannot access PSUM (BIR verifier rule) -> DVE only then
        if psum or self.load["dve"] + cd <= self.load["pool"] + cp:
            self.load["dve"] += cd
            return self.nc.vector
        self.load["pool"] += cp
        return self.nc.gpsimd

    def tt(self, out, a, b, op, free, psum=False):
        cd = (1.0417 * free + 125) if psum else (0.52 * free + 60)
        eng = self._choose(cd, 0.8333 * free + 120, psum)
        eng.tensor_tensor(out, a, b, op)

    def tsp(self, out, a, scal, op, free, psum=False):
        cd = (1.0417 * free + 125) if psum else (0.26 * free + 60)
        eng = self._choose(cd, 0.8333 * free + 120, psum)
        eng.tensor_scalar(out=out, in0=a, scalar1=scal, scalar2=None, op0=op)

    def stt(self, out, a, scal, b, op0, op1, free, psum=False):
        cd = (1.0417 * free + 125) if psum else (1.0417 * free + 60)
        eng = self._choose(cd, 0.8333 * free + 120, psum)
        eng.scalar_tensor_tensor(out=out, in0=a, scalar=scal, in1=b,
                                 op0=op0, op1=op1)

    def cp(self, dst, src, free, psum=False):
        cd = (1.0417 * free + 125) if psum else (0.26 * free + 60)
        eng = self._choose(cd, 0.8333 * free + 120, psum)
        eng.tensor_copy(dst, src)


def build_program(cfg: KCfg = FULL):
    nc = bacc.Bacc("TRN2", target_bir_lowering=False)

    def dram(name, shape, dt=BF, out=False):
        return nc.declare_dram_parameter(name, list(shape), dt, isOutput=out)

    D = {
        # g-major: ch 4g+(qa1, qa1s, ka1, ka1s)
        "r1in": dram("r1in", (16, P, T)),
        "tabr1": dram("tabr1", (4, P, T)),     # c1q s1q c1 s1 (signed)
        # packed half-partition: rows 0:64 sigma32 data, rows 64:128 raw data
        "d64q": dram("d64q", (8, P, T)),       # ch j = B-head j d64 slice
        "d64k": dram("d64k", (4, P, T)),       # ch g = B-kv g d64 slice
        "t64": dram("t64", (4, P, T)),         # c64q s64q c64k s64k (dup halves)
        "qa2": dram("qa2", (8, P, T)),
        "ka2": dram("ka2", (8, P, T)),
        "tabm": dram("tabm", (6, P, T)),       # c2q s2q c2k s2k cBq sBq
        "vcat": dram("vcat", (T, 1792)),       # va1 512 | va2 1024 | vb1 256
        "consts": dram("consts", (2, P, P)),   # tri(NEG strict upper), iden
        "wvec": dram("wvec", (P, 4), dt=F32),
    }
    outQ = dram("outQ", (T, 1024), out=True)
    r1in_r = D["r1in"].rearrange("c p t -> p c t")
    qa2_r = D["qa2"].rearrange("c p t -> p c t")
    ka2_r = D["ka2"].rearrange("c p t -> p c t")
    d64q_r = D["d64q"].rearrange("c p t -> p c t")
    d64k_r = D["d64k"].rearrange("c p t -> p c t")
    vcat_r = D["vcat"].rearrange("(c p) d -> p c d", p=P)

    with ExitStack() as ctx:
        tc = ctx.enter_context(tile.TileContext(nc))
        pers = ctx.enter_context(tc.tile_pool(name="pers", bufs=1))
        pick = _Pick(nc)

        # ---------------- persistent tiles ----------------
        r1q = pers.tile([P, 4, T], BF, name="r1q")
        r1k = pers.tile([P, 4, T], BF, name="r1k")
        tabm = pers.tile([P, 6, T], BF, name="tabm")
        cst = pers.tile([P, 2, P], BF, name="cst")
        wv = pers.tile([P, 4], F32, name="wv")
        t64t = pers.tile([P, 4, T], BF, name="t64t")
        d64qt = pers.tile([P, 8, T], BF, name="d64qt")
        d64kt = pers.tile([P, 4, T], BF, name="d64kt")
        tri, iden = cst[:, 0, :], cst[:, 1, :]

        tabr1 = pers.tile([P, 4, T], BF, name="tabr1")

        # ---------------- work pools ----------------
        mixp = ctx.enter_context(tc.tile_pool(name="mix", bufs=2))
        scr = ctx.enter_context(tc.tile_pool(name="scr", bufs=1))
        ptp = ctx.enter_context(tc.tile_pool(name="pt", bufs=2))
        tAp = ctx.enter_context(tc.tile_pool(name="tA", bufs=2))
        outp = ctx.enter_context(tc.tile_pool(name="out", bufs=1))
        spsum = ctx.enter_context(tc.tile_pool(name="sp", bufs=2, space="PSUM"))
        ypsum = ctx.enter_context(tc.tile_pool(name="yp", bufs=4, space="PSUM"))

        c2q, s2q = tabm[:, 0, :], tabm[:, 1, :]
        c2k, s2k = tabm[:, 2, :], tabm[:, 3, :]
        cBq, sBq = tabm[:, 4, :], tabm[:, 5, :]

        state = {}

        def prefetch(g, skip_r1=False):
            if g >= 4 or ("ka2c", g) in state:
                return
            if not skip_r1:
                r1g = mixp.tile([P, 4, T], BF, tag="r1g", name="r1g", bufs=1)
                nc.sync.dma_start(out=r1g, in_=r1in_r[:, 4 * g:4 * g + 4, :])
                state[("r1g", g)] = r1g
                qa2c = mixp.tile([P, 2, T], BF, tag="qa2c", name="qa2c")
                nc.sync.dma_start(out=qa2c, in_=qa2_r[:, 2 * g:2 * g + 2, :])
                state[("qa2c", g)] = qa2c
            ka2c = mixp.tile([P, 2, T], BF, tag="ka2c", name="ka2c")
            nc.sync.dma_start(out=ka2c, in_=ka2_r[:, 2 * g:2 * g + 2, :])
            va2g = mixp.tile([P, TK, 256], BF, tag="va2g", name="va2g")
            nc.sync.dma_start(out=va2g, in_=vcat_r[:, :, 512 + 256 * g:768 + 256 * g])
            va1g = mixp.tile([P, TK, P], BF, tag="va1g", name="va1g")
            nc.sync.dma_start(out=va1g, in_=vcat_r[:, :, P * g:P * g + P])
            vb1g = mixp.tile([P, TK, 64], BF, tag="vb1g", name="vb1g")
            nc.sync.dma_start(out=vb1g, in_=vcat_r[:, :, 1536 + 64 * g:1600 + 64 * g])
            if g >= 1:
                nc.sync.dma_start(out=d64qt[:, 2 * g:2 * g + 2, :],
                                  in_=d64q_r[:, 2 * g:2 * g + 2, :])
                nc.sync.dma_start(out=d64kt[:, g:g + 1, :],
                                  in_=d64k_r[:, g:g + 1, :])
            state[("ka2c", g)] = ka2c
            state[("va2", g)] = va2g
            state[("va1", g)] = va1g
            state[("vb1", g)] = vb1g

        def r1build(g):
            r1g = state[("r1g", g)]
            u1 = scr.tile([P, T], BF, tag="u1", name="u1")
            pick.tt(u1, r1g[:, 1, :], tabr1[:, 1, :], mult, T)
            pick.tt(r1q[:, g, :], r1g[:, 0, :], tabr1[:, 0, :], mult, T)
            pick.tt(r1q[:, g, :], r1q[:, g, :], u1, add, T)
            pick.tt(u1, r1g[:, 3, :], tabr1[:, 3, :], mult, T)
            pick.tt(r1k[:, g, :], r1g[:, 2, :], tabr1[:, 2, :], mult, T)
            pick.tt(r1k[:, g, :], r1k[:, g, :], u1, add, T)

        # group-0 critical-path loads, consumption-ordered on the SP queue
        r1g0 = mixp.tile([P, 4, T], BF, tag="r1g", name="r1g0", bufs=1)
        nc.sync.dma_start(out=r1g0, in_=r1in_r[:, 0:4, :])
        state[("r1g", 0)] = r1g0
        nc.sync.dma_start(out=tabr1, in_=D["tabr1"].rearrange("c p t -> p c t"))
        qa2c0 = mixp.tile([P, 2, T], BF, tag="qa2c", name="qa2c0")
        nc.sync.dma_start(out=qa2c0, in_=qa2_r[:, 0:2, :])
        nc.sync.dma_start(out=tabm, in_=D["tabm"].rearrange("c p t -> p c t"))
        nc.sync.dma_start(out=cst, in_=D["consts"].rearrange("c p t -> p c t"))
        nc.sync.dma_start(out=wv, in_=D["wvec"][:, :])
        state[("qa2c", 0)] = qa2c0
        prefetch(0, skip_r1=True)
        nc.sync.dma_start(out=t64t, in_=D["t64"].rearrange("c p t -> p c t"))
        nc.sync.dma_start(out=d64qt[:, 0:2, :], in_=d64q_r[:, 0:2, :])
        nc.sync.dma_start(out=d64kt[:, 0:1, :], in_=d64k_r[:, 0:1, :])

        def mix_A(xt, ch, cpos, spos, r1, tag):
            """[P,2,T] mix for config-A (d=256 rope + folded d128 part)."""
            qm = mixp.tile([P, 2, T], BF, tag=tag, name=tag)
            u = scr.tile([P, T], BF, tag="uA", name="uA")
            x0, x1 = xt[:, ch, :], xt[:, ch + 1, :]
            pick.tt(u, x1, spos, mult, T)
            pick.tt(qm[:, 0, :], x0, cpos, mult, T)
            pick.tt(qm[:, 0, :], qm[:, 0, :], u, sub, T)
            pick.tt(qm[:, 0, :], qm[:, 0, :], r1, add, T)
            pick.tt(u, x0, spos, mult, T)
            pick.tt(qm[:, 1, :], x1, cpos, mult, T)
            pick.tt(qm[:, 1, :], qm[:, 1, :], u, add, T)
            return qm

        def r64build(src, ch, ctab, stab, dst, dch):
            """dst[0:64, dch] = rope64 of packed src channel ch."""
            u = scr.tile([P, T], BF, tag="u64", name="u64")
            pick.tt(dst[0:64, dch, :], src[0:64, ch, :], stab[0:64, :], mult, T)
            pick.tt(u[64:P, :], src[64:P, ch, :], ctab[64:P, :], mult, T)
            pick.cp(u[0:64, :], u[64:P, :], T)
            pick.tt(dst[0:64, dch, :], dst[0:64, dch, :], u[0:64, :], add, T)

        def phase1(qm_chunks, km_chunks, pt):
            ndc = len(qm_chunks)
            for c in range(TK):
                q0 = P * c
                sT = spsum.tile([P, T], F32, tag="sT", name="sT")
                pieces = ([(q0, 512), (512, T)] if c < 4 else [(q0, T)])
                for (a, b) in pieces:
                    for dc in range(ndc):
                        nc.tensor.matmul(sT[:, a:b],
                                         km_chunks[dc][:, q0:q0 + P],
                                         qm_chunks[dc][:, a:b],
                                         start=(dc == 0), stop=(dc == ndc - 1))
                nc.tensor.matmul(sT[:, q0:q0 + P], tri, iden,
                                 start=False, stop=True, skip_group_check=True)
                nc.scalar.activation(pt[:, c, q0:T], sT[:, q0:T], Exp)

        def phase2_A(pt, vm, tA):
            rec = scr.tile([P, TK], F32, tag="recA", name="recA")
            for qc in range(TK):
                y = ypsum.tile([P, 512], F32, tag="y", name="y")
                for c in range(qc + 1):
                    nc.tensor.matmul(y[:, 0:257],
                                     pt[:, c, P * qc:P * qc + P],
                                     vm[:, c, :],
                                     start=(c == 0), stop=(c == qc))
                nc.vector.reciprocal(rec[:, qc:qc + 1], y[:, 256:257])
                nc.scalar.activation(tA[:, qc, :], y[:, 0:256],
                                     mybir.ActivationFunctionType.Copy,
                                     scale=rec[:, qc:qc + 1])

        def phase2_B(pt, vm, tA, outt, hh):
            rec = scr.tile([P, TK], F32, tag="recB", name="recB")
            for qc in range(TK):
                y = ypsum.tile([P, 512], F32, tag="y", name="y")
                for c in range(qc + 1):
                    nc.tensor.matmul(y[:, 0:129],
                                     pt[:, c, P * qc:P * qc + P],
                                     vm[:, c, :],
                                     start=(c == 0), stop=(c == qc))
                nc.vector.reciprocal(rec[:, qc:qc + 1], y[:, 128:129])
                pick.stt(outt[:, qc, 128 * hh:128 * hh + 128],
                         y[:, 0:128], rec[:, qc:qc + 1],
                         tA[:, qc, 128 * hh:128 * hh + 128],
                         mult, add, 128, psum=True)

        def do_A(g):
            r1build(g)
            prefetch(g + 1)
            qm = mix_A(state[("qa2c", g)], 0, c2q, s2q, r1q[:, g, :], "qmA")
            km = mix_A(state[("ka2c", g)], 0, c2k, s2k, r1k[:, g, :], "kmA")
            vm = mixp.tile([P, TK, 257], BF, tag="vmA", name="vmA", bufs=1)
            uv = scr.tile([P, TK, P], BF, tag="uvA", name="uvA")
            pick.tsp(vm[:, :, 0:256], state[("va2", g)], wv[:, 1:2], mult, 2048)
            pick.tsp(uv, state[("va1", g)], wv[:, 0:1], mult, 1024)
            pick.tt(vm[:, :, 0:P], vm[:, :, 0:P], uv, add, 1024)
            nc.vector.memset(vm[:, :, 256:257], 1.0)
            pt = ptp.tile([P, TK, T], BF, tag="pt", name="ptA")
            phase1([qm[:, 0, :], qm[:, 1, :]], [km[:, 0, :], km[:, 1, :]], pt)
            tA = tAp.tile([P, TK, 256], BF, tag="tA", name="tA")
            phase2_A(pt, vm, tA)
            state[g] = tA

        def do_B(h):
            g, hh = h // 2, h % 2
            if hh == 0:
                r64g = mixp.tile([64, 2, T], BF, tag="r64q", name="r64q")
                r64build(d64qt, 2 * g, t64t[:, 0, :], t64t[:, 1, :], r64g, 0)
                r64build(d64qt, 2 * g + 1, t64t[:, 0, :], t64t[:, 1, :], r64g, 1)
                r64kg = mixp.tile([64, 1, T], BF, tag="r64k", name="r64k")
                r64build(d64kt, g, t64t[:, 2, :], t64t[:, 3, :], r64kg, 0)
                km = mixp.tile([P, T], BF, tag="kmB", name="kmB")
                pick.tt(km[0:64, :], r1k[0:64, g, :], r64kg[:, 0, :], add, T)
                pick.cp(km[64:P, :], r1k[64:P, g, :], T)
                vm = mixp.tile([P, TK, 129], BF, tag="vmB", name="vmB", bufs=1)
                uv = scr.tile([P, TK, 64], BF, tag="uvB", name="uvB")
                pick.tsp(vm[:, :, 0:128], state[("va1", g)], wv[:, 3:4], mult, 1024)
                pick.tsp(uv, state[("vb1", g)], wv[:, 2:3], mult, 512)
                pick.tt(vm[:, :, 0:64], vm[:, :, 0:64], uv, add, 512)
                nc.vector.memset(vm[:, :, 128:129], 1.0)
                state[("B", g)] = (km, vm, r64g)
                outt = outp.tile([P, TK, 256], BF, tag="outt", name="outt")
                state[("o", g)] = outt
            km, vm, r64g = state[("B", g)]
            outt = state[("o", g)]
            qm = mixp.tile([P, T], BF, tag="qmB", name="qmB")
            u = scr.tile([P, T], BF, tag="uB", name="uB")
            sg = scr.tile([P, T], BF, tag="sgB", name="sgB")
            qa2c = state[("qa2c", g)]
            pick.cp(sg[0:64, :], qa2c[64:P, hh, :], T)
            pick.cp(sg[64:P, :], qa2c[0:64, hh, :], T)
            pick.tt(u, sg, sBq, mult, T)
            pick.tt(qm, qa2c[:, hh, :], cBq, mult, T)
            pick.tt(qm, qm, u, add, T)
            pick.tt(qm[0:64, :], qm[0:64, :], r64g[:, hh, :], add, T)
            pt = ptp.tile([P, TK, T], BF, tag="pt", name="ptB")
            phase1([qm], [km], pt)
            phase2_B(pt, vm, state[g], outt, hh)
            if hh == 1:
                nc.sync.dma_start(
                    out=outQ.rearrange("(c p) d -> p c d", p=P)
                    [:, :, 256 * g:256 * g + 256],
                    in_=outt)

        for g in range(4):
            do_A(g)
            do_B(2 * g)
            do_B(2 * g + 1)

    nc.compile()
    return nc


# ---------------------------------------------------------------------------
# Host side
# ---------------------------------------------------------------------------

def _rope_tabs(pos, d, scale=1.0):
    """cos/sin tables [d, T]; sin SIGNED math-order (rows < d/2 negated)."""
    inv = 1.0 / (10000.0 ** (np.arange(0, d, 2, dtype=np.float32) / d))
    ang = inv[:, None] * pos[None, :].astype(np.float32)
    ang = np.concatenate([ang, ang], 0)
    c = (scale * np.cos(ang)).astype(np.float32)
    s = (scale * np.sin(ang)).astype(np.float32)
    s[: d // 2] *= -1.0
    return c, s


def _sigma(x, half):
    sh = x.shape
    y = x.reshape(-1, 2, half, *sh[1:])
    return np.ascontiguousarray(y[:, ::-1].reshape(sh))


def make_core_inputs(q, k, v, pos, weights, s, cfg: KCfg = FULL):
    """q,k,v: [T, 2048] fp32 for one batch; returns per-core input dict."""
    bf = lambda x: np.ascontiguousarray(x, dtype=NPBF)
    w0, w1, w2, w3 = [float(x) for x in weights]
    fA2 = 1.0 / 16.0
    fB2 = 1.0 / math.sqrt(128.0)

    qa1 = q[:, 512 * s:512 * s + 512].T          # [512, T]
    qa2 = q[:, 1024 * s:1024 * s + 1024].T       # [1024, T]
    ka1 = k[:, 512 * s:512 * s + 512].T
    ka2 = k[:, 1024 * s:1024 * s + 1024].T
    kb1 = k[:, 256 * s:256 * s + 256].T          # [256, T]

    qa1b = qa1.reshape(4, P, T)
    qa1s = _sigma(qa1, 64).reshape(4, P, T)
    ka1b = ka1.reshape(4, P, T)
    ka1s = _sigma(ka1, 64).reshape(4, P, T)
    r1in = np.stack([qa1b, qa1s, ka1b, ka1s], 1).reshape(16, P, T)  # g-major

    c1q, s1q = _rope_tabs(pos, 128, fA2 * w0 * w0)
    c1, s1 = _rope_tabs(pos, 128)
    tabr1 = np.stack([c1q, s1q, c1, s1])

    # packed d64: rows 0:64 sigma32 data, rows 64:128 raw data
    dq = qa1.reshape(8, 64, T)
    dqs = _sigma(qa1, 32).reshape(8, 64, T)
    d64q = np.concatenate([dqs, dq], 1)                       # [8, 128, T]
    dk = kb1.reshape(4, 64, T)
    dks = _sigma(kb1, 32).reshape(4, 64, T)
    d64k = np.concatenate([dks, dk], 1)                       # [4, 128, T]

    c64q, s64q = _rope_tabs(pos, 64, fB2 * w3 * w2)
    c64k, s64k = _rope_tabs(pos, 64, w2 / w3)
    t64 = np.stack([np.concatenate([c64q, c64q], 0),
                    np.concatenate([s64q, s64q], 0),
                    np.concatenate([c64k, c64k], 0),
                    np.concatenate([s64k, s64k], 0)])         # [4, 128, T]

    c2q, s2q = _rope_tabs(pos, 256, fA2 * w0 * w1)
    c2k, s2k = _rope_tabs(pos, 256, w1 / w0)
    cBq, sBq = _rope_tabs(pos, 128, fB2 * w3 * w3)
    tabm = np.stack([c2q[:P], -s2q[:P], c2k[:P], -s2k[:P], cBq, sBq])

    vcat = np.concatenate([v[:, 512 * s:512 * s + 512],
                           v[:, 1024 * s:1024 * s + 1024],
                           v[:, 256 * s:256 * s + 256]], 1)   # [T, 1792]

    tri = np.zeros((P, P), np.float32)
    j, kk = np.mgrid[0:P, 0:P]
    tri[j < kk] = NEG
    consts = np.stack([tri, np.eye(P, dtype=np.float32)])

    arrs = {
        "r1in": bf(r1in), "tabr1": bf(tabr1), "d64q": bf(d64q),
        "d64k": bf(d64k), "t64": bf(t64),
        "qa2": bf(qa2.reshape(8, P, T)),
        "ka2": bf(ka2.reshape(8, P, T)), "tabm": bf(tabm),
        "vcat": bf(vcat), "consts": bf(consts),
        "wvec": np.tile(np.asarray(weights, np.float32)[None, :], (P, 1)),
    }
    del d64q, d64k  # noqa
    return arrs


_PROGRAM_CACHE = {}
TRACE = False
LAST_RESULT = None


def kernel(q_m, k_m, v_m, weights, attention_mask, position_ids):
    global LAST_RESULT
    from concourse.bass_utils import run_bass_kernel_spmd

    cfg = FULL
    q_m = np.asarray(q_m, np.float32)
    k_m = np.asarray(k_m, np.float32)
    v_m = np.asarray(v_m, np.float32)
    weights = np.asarray(weights, np.float32)
    attention_mask = np.asarray(attention_mask, np.float32)
    position_ids = np.asarray(position_ids)
    B, Tq, H = q_m.shape

    causal = np.where(np.tril(np.ones((Tq, Tq), bool)), 0.0, NEG).astype(np.float32)
    for b in range(B):
        assert np.array_equal(attention_mask[b, 0], causal), "non-causal mask"

    if "nc" not in _PROGRAM_CACHE:
        _PROGRAM_CACHE["nc"] = build_program(cfg)
    nc = _PROGRAM_CACHE["nc"]

    in_maps = []
    for b in range(B):
        for s in range(2):
            in_maps.append(make_core_inputs(
                q_m[b], k_m[b], v_m[b], position_ids[b], weights, s, cfg))
    res = run_bass_kernel_spmd(nc, in_maps, list(range(8)), trace=TRACE)
    LAST_RESULT = res
    out = np.zeros((B, Tq, H), np.float32)
    for b in range(B):
        for s in range(2):
            out[b, :, 1024 * s:1024 * s + 1024] = \
                res.results[2 * b + s]["outQ"].astype(np.float32)
    return out
